# revision 2
# baseline (speedup 1.0000x reference)
"""Trainium2 Bass kernel for nn_LmLSTM: embedding -> 2x masked LSTM -> vocab projection.

v2 redesign vs baseline:
- Feature-major wire format for the per-step AllGather: each rank ships
  [128 features, 32 (h1|h0)] bf16 directly from SBUF state (no PE transposes,
  no staging copies on either side). Receive lands h0 into a pooled tile and
  h1 straight into the projection stage ring via two strided DMAs.
- x @ Wx0 (+b0) hoisted out of the recurrence: computed as 32 batched GEMM
  slots interleaved into early iterations; in-loop l0 gates start from one
  identity matmul per gate instead of 4 x-chunk matmuls + bias.
- Cell math: 2 activation calls (TANH on g-groups, one wide SIG on the rest),
  h produced directly in bf16 and masked via copy_predicated into a
  persistent bf16 send tile.
- Output logits in bf16; bout added on device; the (rare) token==0 rows are
  fixed up on host (reference semantics: masked rows = onehot0).

Sharding: gate-sharded recurrence (core r owns hidden slice [128r,128r+128) of
both layers); [H,V] projection vocab-sharded (4000 cols/core), tokens laid out
(t, b)-major so projection chunks stream during the recurrence AG waits.
"""

import os
import sys
import types

import numpy as np
import ml_dtypes

# ---------------------------------------------------------------------------
# Environment shims (self-contained): NTFF profile hook + walrus wait-split.
# ---------------------------------------------------------------------------


def _install_axon_profile_hook():
    if "antenv.axon_hooks" in sys.modules:
        return
    holder = [None]
    mod = types.ModuleType("antenv.axon_hooks")
    mod.set_axon_ntff_profile_hook = lambda h: holder.__setitem__(0, h)
    mod.get_axon_ntff_profile_hook = lambda: holder[0]
    sys.modules["antenv.axon_hooks"] = mod
    try:
        import antenv

        antenv.axon_hooks = mod
        from trn_agent_boot.trn_boot import _ntff_profile_via_ctypes

        mod.set_axon_ntff_profile_hook(
            _ntff_profile_via_ctypes("/opt/axon/libaxon_pjrt.so")
        )
    except Exception:
        pass


_install_axon_profile_hook()

import concourse.bass as bass  # noqa: E402
import concourse.mybir as mybir  # noqa: E402
import concourse.tile as tile  # noqa: E402
from concourse.bass_utils import run_bass_kernel_spmd  # noqa: E402


def _install_wait_split():
    """This container's walrus accepts at most one sem-wait per instruction.
    Hoist excess waits onto same-engine nops placed just before."""
    if getattr(bass.Bass, "_waitsplit_installed", False):
        return
    counter = [0]

    def _split(m):
        for f in m.functions:
            for bb in f.blocks:
                il = bb.instructions
                if not any(
                    i.sync_info is not None and len(i.sync_info.on_wait) > 1
                    for i in il
                ):
                    continue
                new = []
                for inst in il:
                    si = inst.sync_info
                    if si is not None and len(si.on_wait) > 1:
                        waits = list(si.on_wait)
                        si.on_wait = waits[:1]
                        for w in waits[1:]:
                            counter[0] += 1
                            nop = mybir.InstNoOp(
                                name=f"waitsplit_{counter[0]}", ins=[], outs=[]
                            )
                            nop.engine = inst.engine
                            nop.sync_info = mybir.SyncInfo(
                                on_wait=[w], on_update=[]
                            )
                            new.append(nop)
                    new.append(inst)
                il.clear()
                il.extend(new)

    orig = bass.Bass.to_json_bytes

    def patched(self, *a, **kw):
        _split(self.m)
        return orig(self, *a, **kw)

    bass.Bass.to_json_bytes = patched
    bass.Bass._waitsplit_installed = True


_install_wait_split()

# ---------------------------------------------------------------------------
# Problem constants
# ---------------------------------------------------------------------------
V, E, H = 32000, 512, 1024
B = 16
T = int(os.environ.get("KERNEL_T", "256"))
NC = 8
VS = V // NC  # 4000 vocab cols per core
NTOK = B * T
NTC = NTOK // 128  # token chunks (8 t-steps x 16 b each)
SB = 16  # batch width
NSZ = VS // 8  # 500 vocab cols per projection n-group
XW = min(512, T * 16)  # xz0 GEMM slot width
NXC = (T * 16) // XW
F32 = mybir.dt.float32
BF16 = mybir.dt.bfloat16
U8 = mybir.dt.uint8
SIG = mybir.ActivationFunctionType.Sigmoid
TANH = mybir.ActivationFunctionType.Tanh
COPY = mybir.ActivationFunctionType.Copy

# psum gate-group layout (col = group*SB): [g1 g0 i1 i0 f1 f0 o1 o0]
# group index = 2*gate + (0 if layer1 else 1), packed gate order (g,i,f,o)


def build_nc():
    nc = bass.Bass()
    d_w0 = nc.dram_tensor("w0p", [128, 12 * 4 * 128], BF16, kind="ExternalInput")
    d_w1 = nc.dram_tensor("w1p", [128, 16 * 4 * 128], BF16, kind="ExternalInput")
    d_wout = nc.dram_tensor("woutp", [128, 8 * VS], BF16, kind="ExternalInput")
    d_b1bc = nc.dram_tensor("b1bc", [128, 4 * 16], BF16, kind="ExternalInput")
    d_b0row = nc.dram_tensor("b0row", [1, 4 * 128], BF16, kind="ExternalInput")
    d_ones = nc.dram_tensor("onesr", [1, 512], BF16, kind="ExternalInput")
    d_boutr = nc.dram_tensor("boutr", [1, VS], BF16, kind="ExternalInput")
    d_xt = nc.dram_tensor("xt", [E, T * 16], BF16, kind="ExternalInput")
    d_masku = nc.dram_tensor("masku", [128, T * 16], U8, kind="ExternalInput")
    d_identb = nc.dram_tensor("identb", [128, 128], BF16, kind="ExternalInput")
    d_out = nc.dram_tensor("out", [NTOK, VS], BF16, kind="ExternalOutput")

    rg = [list(range(NC))]

    with tile.TileContext(nc) as tc:
        with (
            tc.tile_pool(name="wp", bufs=1) as wp,
            tc.tile_pool(name="sp", bufs=4) as sp,
            tc.tile_pool(name="hp", bufs=3) as hp,
            tc.tile_pool(name="pp", bufs=2, space="PSUM") as pp,
            tc.tile_pool(name="qq", bufs=2, space="PSUM") as qq,
            tc.tile_pool(name="dp", bufs=3, space="DRAM") as dp,
        ):
            # ---- persistent loads ----
            w0t = wp.tile([128, 12 * 4 * 128], BF16, tag="w0t")
            w1t = wp.tile([128, 16 * 4 * 128], BF16, tag="w1t")
            woutt = wp.tile([128, 8 * VS], BF16, tag="woutt")
            b1bc = wp.tile([128, 4 * 16], BF16, tag="b1bc")
            b0rt = wp.tile([1, 4 * 128], BF16, tag="b0rt")
            onest = wp.tile([1, 512], BF16, tag="onest")
            boutrt = wp.tile([1, VS], BF16, tag="boutrt")
            masku = wp.tile([128, T * 16], U8, tag="masku")
            identb = wp.tile([128, 128], BF16, tag="identb")
            nc.gpsimd.dma_start(identb[:], d_identb[:])
            nc.gpsimd.dma_start(b1bc[:], d_b1bc[:])
            nc.gpsimd.dma_start(b0rt[:], d_b0row[:])
            nc.gpsimd.dma_start(onest[:], d_ones[:])
            nc.gpsimd.dma_start(boutrt[:], d_boutr[:])
            nc.gpsimd.dma_start(masku[:], d_masku[:])
            nc.scalar.dma_start(w0t[:], d_w0[:])
            xt = []
            for k in range(4):
                xk = wp.tile([128, T * 16], BF16, tag=f"xt{k}")
                eng = nc.scalar if k < 2 else nc.sync
                eng.dma_start(xk[:], d_xt[128 * k : 128 * (k + 1), :])
                xt.append(xk)
            nc.sync.dma_start(w1t[:], d_w1[:])
            nc.gpsimd.dma_start(woutt[:], d_wout[:])

            # hoisted l0 input projection (+b0): [128 gatecols, g(4) x tok]
            xz0 = wp.tile([128, 4 * T * 16], BF16, tag="xz0")

            # projection stage rings: [128, k(8) j(8) b(16)] bf16, ping-pong
            stage0 = wp.tile([128, 8 * 8 * 16], BF16, tag="stage0")
            stage1 = wp.tile([128, 8 * 8 * 16], BF16, tag="stage1")
            stages = [stage0, stage1]

            # persistent state: c (f32) and send h (bf16), cols = [l1 | l0]
            cst = wp.tile([128, 2 * SB], F32, tag="cst")
            hbs = wp.tile([128, 2 * SB], BF16, tag="hbs")
            nc.vector.memset(cst[:], 0.0)
            nc.vector.memset(hbs[:], 0.0)

            # ---------------- xz0 hoist slots -----------------------------
            def emit_xz0(g, tcn):
                ps = qq.tile([128, XW], F32, tag="xz", bufs=2)
                nc.tensor.matmul(
                    ps[:],
                    b0rt[:1, g * 128 : (g + 1) * 128],
                    onest[:1, 0:XW],
                    start=True,
                    stop=False,
                )
                for q in range(4):
                    nc.tensor.matmul(
                        ps[:],
                        w0t[:, (q * 4 + g) * 128 : (q * 4 + g + 1) * 128],
                        xt[q][:, tcn * XW : (tcn + 1) * XW],
                        start=False,
                        stop=(q == 3),
                    )
                nc.scalar.activation(
                    xz0[:, g * (T * 16) + tcn * XW : g * (T * 16) + (tcn + 1) * XW],
                    ps[:],
                    COPY,
                )

            # ---------------- projection ----------------------------------
            def emit_proj(tcn, n):
                ps = qq.tile([128, NSZ], F32, tag="proj", bufs=2)
                stg = stages[tcn % 2]
                nc.tensor.matmul(
                    ps[:],
                    onest[:1, 0:128],
                    boutrt[:1, n * NSZ : (n + 1) * NSZ],
                    start=True,
                    stop=False,
                )
                for k in range(8):
                    nc.tensor.matmul(
                        ps[:],
                        stg[:, 128 * k : 128 * (k + 1)],
                        woutt[:, k * VS + n * NSZ : k * VS + (n + 1) * NSZ],
                        start=False,
                        stop=(k == 7),
                    )
                lg = sp.tile([128, NSZ], BF16, tag="lg")
                nc.vector.tensor_copy(lg[:], ps[:])
                nc.gpsimd.dma_start(
                    d_out[128 * tcn : 128 * (tcn + 1), n * NSZ : (n + 1) * NSZ],
                    lg[:],
                )

            # proj (tcn, n) emitted at iteration 8*tcn + 11 + n: one extra
            # iteration of slack past the stage-slot landing so the pn=0
            # chunk never stalls the PE FIFO on a fresh stage copy
            proj_sched = {}
            for tcn in range(NTC):
                for n in range(8):
                    proj_sched.setdefault(8 * tcn + 11 + n, []).append((tcn, n))
            proj_done = set()

            # xz0 slot (g, tc): tc=0 in prologue; rest 2 per iteration
            xz0_sched = {}
            slots = [(g, tc) for tc in range(1, NXC) for g in range(4)]
            for i, s in enumerate(slots):
                xz0_sched.setdefault(1 + i // 2, []).append(s)

            # ---------------- recurrence ----------------------------------
            # iteration n computes h0(n) (n<T) and h1(n-1) (n>=1); ships
            # AG(n) = [h1(n-1) | h0(n)] feature-major bf16 [128, 32].
            cc_outs = [None] * (T + 1)

            def receive(n):
                # AG(n-1) -> two half DMAs (64B lines) on the two HWDGE
                # rings so rank 0-3 chunks land/consume while 4-7 transfer;
                # h1(n-2) then copied into the projection stage ring by DVE
                cco = cc_outs[n - 1]
                ccv = cco.rearrange("(r p) c -> p r c", p=128)
                hbA = hp.tile([128, 4 * 2 * SB], BF16, tag="hbA")
                hbB = hp.tile([128, 4 * 2 * SB], BF16, tag="hbB")
                nc.sync.dma_start(
                    hbA.rearrange("p (r c) -> p r c", r=4), ccv[:, 0:4, :]
                )
                nc.scalar.dma_start(
                    hbB.rearrange("p (r c) -> p r c", r=4), ccv[:, 4:8, :]
                )
                hvA = hbA.rearrange("p (r l b) -> p r l b", r=4, l=2)
                hvB = hbB.rearrange("p (r l b) -> p r l b", r=4, l=2)

                def hv(k, l):
                    return hvA[:, k, l, :] if k < 4 else hvB[:, k - 4, l, :]

                if n >= 2:
                    m = n - 2
                    stg = stages[(m // 8) % 2]
                    sv = stg.rearrange("p (k j b) -> p k j b", k=8, j=8)
                    nc.vector.tensor_copy(
                        sv[:, 0:4, m % 8, :], hvA[:, :, 0, :]
                    )
                    nc.vector.tensor_copy(
                        sv[:, 4:8, m % 8, :], hvB[:, :, 0, :]
                    )
                return hv

            def emit_substep(n):
                hv = receive(n) if n >= 1 else None
                l0 = n < T
                l1 = n >= 1

                # one PSUM tile per gate ([l1 | l0] cols) so each gate's
                # activation fires as soon as that gate's matmuls drain
                zg = [
                    pp.tile(
                        [128, 2 * SB],
                        F32,
                        tag=f"z{gname}",
                        name=f"zg_{gname}",
                        bufs=1,
                    )
                    for gname in ("g", "i", "f", "o")
                ]

                def g1_group(gate):
                    dst = zg[gate][:, 0:SB]
                    nc.tensor.matmul(
                        dst,
                        identb[:],
                        b1bc[:, gate * 16 : (gate + 1) * 16],
                        start=True,
                        stop=False,
                    )
                    for k in range(8):
                        nc.tensor.matmul(
                            dst,
                            w1t[:, (k * 4 + gate) * 128 : (k * 4 + gate + 1) * 128],
                            hv(k, 1),
                            start=False,
                            stop=False,
                        )
                    for k in range(8):
                        nc.tensor.matmul(
                            dst,
                            w1t[
                                :,
                                ((8 + k) * 4 + gate) * 128 : ((8 + k) * 4 + gate + 1)
                                * 128,
                            ],
                            hv(k, 0),
                            start=False,
                            stop=(k == 7),
                        )

                def g0_group(gate):
                    dst = zg[gate][:, SB : 2 * SB]
                    nc.tensor.matmul(
                        dst,
                        identb[:],
                        xz0[:, gate * (T * 16) + n * 16 : gate * (T * 16) + n * 16 + 16],
                        start=True,
                        stop=(not l1),
                    )
                    if l1:
                        for k in range(8):
                            nc.tensor.matmul(
                                dst,
                                w0t[
                                    :,
                                    ((4 + k) * 4 + gate) * 128 : ((4 + k) * 4 + gate + 1)
                                    * 128,
                                ],
                                hv(k, 1),
                                start=False,
                                stop=(k == 7),
                            )

                # gate order: g first, then i, f, o
                for gate in (0, 1, 2, 3):
                    if l1:
                        g1_group(gate)
                    if l0 and not (n == 0 and gate == 2):
                        g0_group(gate)

                # ---- cell math ----
                # per-gate ACT calls so each fires as soon as its matmul
                # group drains (c-path: g,i,f; o only gates the last mul)
                gt = sp.tile([128, 8 * SB], F32, tag="gt")
                S1, S2, S4, S6, S8 = SB, 2 * SB, 4 * SB, 6 * SB, 8 * SB
                if l0 and l1:
                    nc.scalar.activation(gt[:, 0:S2], zg[0][:], TANH)
                    nc.scalar.activation(gt[:, S2:S4], zg[1][:], SIG)
                    nc.scalar.activation(gt[:, S4:S6], zg[2][:], SIG)
                    nc.scalar.activation(gt[:, S6:S8], zg[3][:], SIG)
                    tmpa = sp.tile([128, S2], F32, tag="tmpa")
                    tmpb = sp.tile([128, S2], F32, tag="tmpb")
                    cn = sp.tile([128, S2], F32, tag="cn")
                    tcn_ = sp.tile([128, S2], F32, tag="tcn")
                    hn = sp.tile([128, S2], BF16, tag="hn")
                    nc.vector.tensor_mul(tmpa[:], gt[:, S2:S4], gt[:, 0:S2])
                    nc.vector.tensor_mul(tmpb[:], gt[:, S4:S6], cst[:])
                    nc.vector.tensor_add(cn[:], tmpa[:], tmpb[:])
                    nc.scalar.activation(tcn_[:], cn[:], TANH)
                    nc.vector.tensor_mul(hn[:], gt[:, S6:S8], tcn_[:])
                    mk = masku[:, (n - 1) * 16 : (n + 1) * 16]
                    nc.vector.copy_predicated(hbs[:], mk, hn[:])
                    nc.vector.copy_predicated(cst[:], mk, cn[:])
                elif l0:
                    # n == 0: layer0 only; c=0 so cn = i*g
                    nc.scalar.activation(gt[:, S1:S2], zg[0][:, SB:S2], TANH)
                    nc.scalar.activation(gt[:, 3 * SB : 4 * SB], zg[1][:, SB:S2], SIG)
                    nc.scalar.activation(gt[:, 7 * SB : 8 * SB], zg[3][:, SB:S2], SIG)
                    cn = sp.tile([128, SB], F32, tag="cn0")
                    tcn_ = sp.tile([128, SB], F32, tag="tcn0")
                    hn = sp.tile([128, SB], BF16, tag="hn0")
                    nc.vector.tensor_mul(cn[:], gt[:, 3 * SB : 4 * SB], gt[:, S1:S2])
                    nc.scalar.activation(tcn_[:], cn[:], TANH)
                    nc.vector.tensor_mul(hn[:], gt[:, 7 * SB : 8 * SB], tcn_[:])
                    mk = masku[:, n * 16 : (n + 1) * 16]
                    nc.vector.copy_predicated(hbs[:, SB:S2], mk, hn[:])
                    nc.vector.copy_predicated(cst[:, SB:S2], mk, cn[:])
                else:
                    # n == T: layer1 only
                    nc.scalar.activation(gt[:, 0:S1], zg[0][:, 0:SB], TANH)
                    nc.scalar.activation(gt[:, S2 : S2 + SB], zg[1][:, 0:SB], SIG)
                    nc.scalar.activation(gt[:, S4 : S4 + SB], zg[2][:, 0:SB], SIG)
                    nc.scalar.activation(gt[:, S6 : S6 + SB], zg[3][:, 0:SB], SIG)
                    tmpa = sp.tile([128, SB], F32, tag="tmpa1")
                    tmpb = sp.tile([128, SB], F32, tag="tmpb1")
                    cn = sp.tile([128, SB], F32, tag="cn1")
                    tcn_ = sp.tile([128, SB], F32, tag="tcn1")
                    hn = sp.tile([128, SB], BF16, tag="hn1")
                    nc.vector.tensor_mul(tmpa[:], gt[:, S2 : S2 + SB], gt[:, 0:S1])
                    nc.vector.tensor_mul(tmpb[:], gt[:, S4 : S4 + SB], cst[:, 0:SB])
                    nc.vector.tensor_add(cn[:], tmpa[:], tmpb[:])
                    nc.scalar.activation(tcn_[:], cn[:], TANH)
                    nc.vector.tensor_mul(hn[:], gt[:, S6 : S6 + SB], tcn_[:])
                    mk = masku[:, (n - 1) * 16 : n * 16]
                    nc.vector.copy_predicated(hbs[:, 0:SB], mk, hn[:])
                    nc.vector.copy_predicated(cst[:, 0:SB], mk, cn[:])

            def emit_send(n):
                cc_in = dp.tile([128, 2 * SB], BF16, tag="cc_in")
                cc_out = dp.tile(
                    [NC * 128, 2 * SB], BF16, tag="cc_out", addr_space="Shared"
                )
                nc.scalar.dma_start(cc_in[:], hbs[:])
                nc.gpsimd.collective_compute(
                    "AllGather",
                    mybir.AluOpType.bypass,
                    ins=[cc_in.opt()],
                    outs=[cc_out.opt()],
                    replica_groups=rg,
                )
                cc_outs[n] = cc_out

            # ---- prologue: xz0 for tc=0 ----
            for g in range(4):
                emit_xz0(g, 0)

            for n in range(T + 1):
                for tcn, pn in proj_sched.get(n, []):
                    emit_proj(tcn, pn)
                    proj_done.add((tcn, pn))
                for g, tc in xz0_sched.get(n, []):
                    emit_xz0(g, tc)
                emit_substep(n)
                emit_send(n)

            # ---- epilogue: land h1(T-1) into stage, finish proj ----
            m = T - 1
            hbT = hp.tile([128, 8 * 2 * SB], BF16, tag="hbT")
            nc.sync.dma_start(
                hbT.rearrange("p (r c) -> p r c", r=8),
                cc_outs[T].rearrange("(r p) c -> p r c", p=128),
            )
            nc.vector.tensor_copy(
                stages[(m // 8) % 2].rearrange("p (k j b) -> p k j b", k=8, j=8)[
                    :, :, m % 8, :
                ],
                hbT.rearrange("p (r l b) -> p r l b", r=8, l=2)[:, :, 0, :],
            )
            for tcn in range(NTC):
                for n in range(8):
                    if (tcn, n) not in proj_done:
                        emit_proj(tcn, n)
    return nc


_NC_CACHE = [None]


def kernel(tokens, emb, Wx0, Wh0, b0, Wx1, Wh1, b1, Wout, bout):
    tokens = np.asarray(tokens)
    toks = tokens.astype(np.int64)
    emb = np.asarray(emb, np.float32)

    x = emb[toks]  # [B,T,E]
    xt = np.ascontiguousarray(x[:, :T].transpose(2, 1, 0).reshape(E, T * B))
    xt = xt.astype(ml_dtypes.bfloat16)  # cols (t, b)

    fm = (toks != 0)[:, :T]  # [B,T] bool
    fm_tb = np.ascontiguousarray(fm.T).reshape(-1)  # (t,b) order
    masku = np.broadcast_to(fm_tb.reshape(1, T * B), (128, T * B)).astype(np.uint8)

    GO = [2, 0, 1, 3]  # packed gate order [g,i,f,o] from original (i,f,g,o)

    def pack(w, nk):
        # w: [nk*128, 512 cols in packed gate order] -> [128, nk*4*128]
        a = np.asarray(w, np.float32).reshape(nk, 128, 4, 128)
        return (
            np.ascontiguousarray(a.transpose(1, 0, 2, 3))
            .reshape(128, nk * 4 * 128)
            .astype(ml_dtypes.bfloat16)
        )

    ones = np.ones((1, 512), ml_dtypes.bfloat16)
    bouta = np.asarray(bout, np.float32)

    in_maps = []
    for r in range(NC):
        sl = np.arange(128 * r, 128 * (r + 1))
        cols = np.concatenate([g * H + sl for g in GO])
        w0 = np.concatenate([np.asarray(Wx0)[:, cols], np.asarray(Wh0)[:, cols]], 0)
        w1 = np.concatenate([np.asarray(Wx1)[:, cols], np.asarray(Wh1)[:, cols]], 0)
        wo = np.asarray(Wout, np.float32)[:, VS * r : VS * (r + 1)]  # [1024, VS]
        woutp = (
            np.ascontiguousarray(wo.reshape(8, 128, VS).transpose(1, 0, 2))
            .reshape(128, 8 * VS)
            .astype(ml_dtypes.bfloat16)
        )
        b0a = np.asarray(b0, np.float32)
        b1a = np.asarray(b1, np.float32)
        # l1 bias broadcast [128, 4*16]: per gate, per-partition bias
        b1bc = (
            np.stack(
                [np.broadcast_to(b1a[g * H + sl][:, None], (128, 16)) for g in GO],
                axis=1,
            )
            .reshape(128, 4 * 16)
            .astype(ml_dtypes.bfloat16)
        )
        b0row = (
            np.concatenate([b0a[g * H + sl] for g in GO])
            .reshape(1, 4 * 128)
            .astype(ml_dtypes.bfloat16)
        )
        vsl = slice(VS * r, VS * (r + 1))
        in_maps.append(
            {
                "w0p": pack(w0, 12),
                "w1p": pack(w1, 16),
                "woutp": woutp,
                "b1bc": b1bc,
                "b0row": b0row,
                "onesr": ones,
                "boutr": bouta[vsl].reshape(1, VS).astype(ml_dtypes.bfloat16),
                "xt": xt,
                "masku": masku,
                "identb": np.eye(128, dtype=ml_dtypes.bfloat16),
            }
        )

    if _NC_CACHE[0] is None:
        _NC_CACHE[0] = build_nc()
    nc = _NC_CACHE[0]

    trace = os.environ.get("KERNEL_TRACE", "0") == "1"
    res = run_bass_kernel_spmd(
        nc, in_maps, core_ids=list(range(NC)), trace=trace
    )
    if trace and res.exec_time_ns is not None:
        print(f"HW exec time: {res.exec_time_ns} ns")

    logits = np.concatenate(
        [np.asarray(res.results[r]["out"], np.float32) for r in range(NC)], axis=1
    )  # [(t,b), V]
    out = np.ascontiguousarray(
        logits.reshape(T, B, V).transpose(1, 0, 2)
    ).astype(np.float32)
    # host fixup: masked (token==0) rows are onehot0 in the reference
    mb, mt = np.nonzero(~fm)
    if len(mb):
        oh = np.zeros((V,), np.float32)
        oh[0] = 1.0
        out[mb, mt, :] = oh
    if T < tokens.shape[1]:
        full = np.zeros((B, tokens.shape[1], V), np.float32)
        full[:, :T] = out
        out = full
    return out
